# revision 14
# baseline (speedup 1.0000x reference)
"""Trainium2 Bass kernel for nn_ASTHL_23742579212955 (gnn_message_passing).

Sharding: poi dim P=10000 -> 1250 rows/core, user dim U=8000 -> 1000/core
across 8 NeuronCores. Each sparse matmul = 4-queue SWDGE dma_gather of
source rows from a DRAM table + one-hot matmul scatter on PE (edges are
host-sorted into 125-row destination windows; per 128-edge chunk the DVE
builds a [128,125] one-hot carrying the edge values, PE accumulates in
PSUM). Activations are all-gathered between layers; the InfoNCE [P,P]
matmul is row-sharded with a ReduceScatter for column sums.
"""

import sys

sys.path.insert(0, "/opt/trn_rl_repo")

import numpy as np

import concourse.bacc as bacc
import concourse.bass as bass
import concourse.mybir as mybir
import concourse.tile as tile
from concourse import bass_utils

F32 = mybir.dt.float32
I16 = mybir.dt.int16
AF = mybir.ActivationFunctionType
OP = mybir.AluOpType

NCORES = 8
P, U, D, B = 10000, 8000, 64, 4096
PSH, USH = P // NCORES, U // NCORES  # 1250, 1000
WIN = 125
NWP, NWU = PSH // WIN, USH // WIN  # 10, 8
BSH = B // NCORES  # 512
TEMP = 0.2


def _wrap16(idx):
    """[N] int -> wrapped int16 [128, N//16] (pos i at [i%16, i//16], tiled)."""
    a = idx.astype(np.int16).reshape(-1, 16).T
    return np.tile(a, (8, 1)).copy()


def prep_matrix(rows, cols, vals, n_dest, n_win):
    """Partition edges by dest shard, sort into WIN-row windows, pad each
    window to uniform C chunks of 128 edges. Returns C + per-core streams."""
    dsh = n_dest // NCORES
    core = rows // dsh
    local = rows - core * dsh
    win = local // WIN
    rloc = local - win * WIN

    per, maxlen = [], 0
    for c in range(NCORES):
        mc = core == c
        cwin, ccols, crloc, cvals = win[mc], cols[mc], rloc[mc], vals[mc]
        order = np.argsort(cwin, kind="stable")
        cwin, ccols, crloc, cvals = cwin[order], ccols[order], crloc[order], cvals[order]
        bounds = np.searchsorted(cwin, np.arange(n_win + 1))
        rows_c = []
        for w in range(n_win):
            s, e = bounds[w], bounds[w + 1]
            rows_c.append((ccols[s:e], crloc[s:e], cvals[s:e]))
            maxlen = max(maxlen, e - s)
        per.append(rows_c)

    C = (maxlen + 127) // 128
    L = C * 128
    idx_s, rloc_s, val_s = [], [], []
    for c in range(NCORES):
        icat = np.zeros((n_win, L), np.int64)
        rcat = np.zeros((n_win, L), np.float32)
        vcat = np.zeros((n_win, L), np.float32)
        for w in range(n_win):
            ccols, crloc, cvals = per[c][w]
            n = len(ccols)
            icat[w, :n] = ccols
            rcat[w, :n] = crloc
            vcat[w, :n] = cvals
        idx_s.append(_wrap16(icat.reshape(-1)))
        rloc_s.append(rcat.reshape(n_win * C, 128).T.copy())
        val_s.append(vcat.reshape(n_win * C, 128).T.copy())
    return C, idx_s, rloc_s, val_s


def build(meta):
    Cg, Ct, Cs, Cu, Cp = meta["Cg"], meta["Ct"], meta["Cs"], meta["Cu"], meta["Cp"]
    nc = bacc.Bacc("TRN2", target_bir_lowering=False, debug=False,
                   num_devices=NCORES, num_swdge_queues=4)

    def din(name, shape, dt=F32):
        return nc.dram_tensor(name, shape, dt, kind="ExternalInput")

    t_pe = din("pe_shard", [PSH, D])
    t_wg = din("w3", [D, 3 * D])
    t_fw = din("fusion_w", [D, 7 * D])
    t_bias = din("biases", [128, 4 * D])
    t_ue = din("ue_shard", [USH, D])
    t_iota = din("iota", [128, 128])
    t_pidx = din("pidx", [128, 1])
    t_ones = din("ones", [128, 1])
    t_uidx = din("uidx", [128, BSH // 16], I16)

    def streams_in(nm, C, nwin):
        n = nwin * C
        return (din(f"{nm}_idx", [128, n * 8], I16),
                din(f"{nm}_rloc", [128, n]),
                din(f"{nm}_val", [128, n]))

    stream_ins = {
        "geo": streams_in("geo", Cg, NWP),
        "tar": streams_in("tar", Ct, NWP),
        "src": streams_in("src", Cs, NWP),
        "up": streams_in("up", Cu, NWU),
        "pu": streams_in("pu", Cp, NWP),
    }

    t_fp_out = nc.dram_tensor("fusion_pois", [PSH, D], F32, kind="ExternalOutput")
    t_bu_out = nc.dram_tensor("batch_users", [BSH, D], F32, kind="ExternalOutput")
    t_loss_out = nc.dram_tensor("loss", [1, 1], F32, kind="ExternalOutput")

    RG = [list(range(NCORES))]

    with tile.TileContext(nc) as tc:
        with (
            tc.tile_pool(name="const", bufs=1) as cp,
            tc.tile_pool(name="act", bufs=1) as ap,
            tc.tile_pool(name="stream", bufs=1) as sp,
            tc.tile_pool(name="gat", bufs=2) as gpool,
            tc.tile_pool(name="oh", bufs=4) as ohp,
            tc.tile_pool(name="work", bufs=3) as wp,
            tc.tile_pool(name="ps", bufs=3, space="PSUM") as pp,
            tc.tile_pool(name="psz", bufs=2, space="PSUM") as ppz,
            tc.tile_pool(name="dram", bufs=1, space="DRAM") as dr,
        ):
            # ---------- constants & streams ----------
            iota = cp.tile([128, 128], F32)
            nc.sync.dma_start(iota[:], t_iota[:])
            ones = cp.tile([128, 1], F32)
            nc.sync.dma_start(ones[:], t_ones[:])
            bias = cp.tile([128, 4 * D], F32)
            nc.sync.dma_start(bias[:], t_bias[:])
            w3 = cp.tile([D, 3 * D], F32)
            nc.sync.dma_start(w3[:], t_wg[:])
            fw = cp.tile([D, 7, D], F32)
            nc.sync.dma_start(fw[:], t_fw.ap().rearrange("k (g d) -> k g d", g=7))
            pidx = cp.tile([128, 1], F32)
            nc.sync.dma_start(pidx[:], t_pidx[:])
            ident = cp.tile([128, 128], F32)
            nc.vector.tensor_scalar(ident[:], iota[:], pidx[:], 1.0,
                                    op0=OP.is_equal, op1=OP.mult)

            sb_streams = {}
            for nm, C, nw in (("geo", Cg, NWP), ("tar", Ct, NWP),
                              ("src", Cs, NWP), ("up", Cu, NWU),
                              ("pu", Cp, NWP)):
                ti, tr, tv = stream_ins[nm]
                i_sb = sp.tile([128, nw * C * 8], I16, name=f"{nm}_idx_sb")
                nc.sync.dma_start(i_sb[:], ti[:])
                r_sb = sp.tile([128, nw * C], F32, name=f"{nm}_rloc_sb")
                nc.sync.dma_start(r_sb[:], tr[:])
                v_sb = sp.tile([128, nw * C], F32, name=f"{nm}_val_sb")
                nc.sync.dma_start(v_sb[:], tv[:])
                sb_streams[nm] = (i_sb, r_sb, v_sb, C)

            uidx_sb = sp.tile([128, BSH // 16], I16)
            nc.sync.dma_start(uidx_sb[:], t_uidx[:])

            pe = ap.tile([WIN, NWP, D], F32)
            nc.sync.dma_start(pe[:], t_pe.ap().rearrange("(m p) d -> p m d", p=WIN))
            ue = ap.tile([WIN, NWU, D], F32)
            nc.sync.dma_start(ue[:], t_ue.ap().rearrange("(m p) d -> p m d", p=WIN))

            # ---------- helpers ----------
            def l2norm(dst, src):
                sq = wp.tile([WIN, D], F32, name="l2_sq", tag="l2sq")
                s = wp.tile([WIN, 1], F32, name="l2_s", tag="l2s")
                nc.vector.scalar_tensor_tensor(
                    sq[:], src, 1.0, src, OP.mult, OP.mult, accum_out=s[:])
                nc.scalar.sqrt(s[:], s[:])
                nc.vector.tensor_scalar_max(s[:], s[:], 1e-12)
                r = wp.tile([WIN, 1], F32, name="l2_r", tag="l2r")
                nc.vector.reciprocal(r[:], s[:])
                nc.vector.tensor_scalar_mul(dst, src, r[:])

            def ag_shard(src_tile, width, name):
                nw = src_tile.shape[1]
                rows = WIN * nw
                bounce = dr.tile([rows, width], F32, name=f"{name}_b")
                nc.sync.dma_start(
                    bounce.rearrange("(m p) d -> p m d", p=WIN), src_tile[:])
                full = dr.tile([rows * NCORES, width], F32, name=f"{name}_f",
                               addr_space="Shared")
                nc.gpsimd.collective_compute(
                    "AllGather", OP.bypass, replica_groups=RG,
                    ins=[bounce.opt()], outs=[full.opt()])
                return full

            def one_hot(nm, r_sb, v_sb, c):
                oh = ohp.tile([128, WIN], F32, name=f"oh_{nm}", tag="oh")
                nc.vector.tensor_scalar(
                    oh[:], iota[:, :WIN], r_sb[:, c:c + 1], v_sb[:, c:c + 1],
                    op0=OP.is_equal, op1=OP.mult)
                return oh

            def spmm(nm, table_ap, elem_step, out_cb, n_d=D, trans3=False):
                i_sb, r_sb, v_sb, C = sb_streams[nm]
                nw = i_sb.shape[1] // (C * 8)
                nidx = C * 128
                cc_cap = 32 if n_d == D else 8
                ncall = -(-C // cc_cap)
                for w in range(nw):
                    if trans3:
                        ps3 = ppz.tile([D, 3 * WIN], F32, name=f"{nm}_ps3",
                                       tag="ps3")
                        pss = [ps3[:, i * WIN:(i + 1) * WIN] for i in range(3)]
                    else:
                        pss = [pp.tile([WIN, n_d], F32, name=f"{nm}_ps",
                                       tag="ps")]
                    j = 0
                    for call in range(ncall):
                        cc = min(cc_cap, C - call * cc_cap)
                        i0 = (w * nidx + call * cc_cap * 128) // 16
                        gout = gpool.tile([128, cc, n_d], F32, name=f"g_{nm}",
                                          tag="g64" if n_d == D else "g192")
                        nc.gpsimd.dma_gather(
                            gout[:], table_ap, i_sb[:, i0:i0 + cc * 8],
                            cc * 128, cc * 128, n_d, elem_step=elem_step,
                            single_packet=False, queue_num=(w * ncall + call) % 4)
                        for jj in range(cc):
                            oh = one_hot(nm, r_sb, v_sb, w * C + j)
                            if trans3:
                                for i in range(3):
                                    nc.tensor.matmul(
                                        pss[i], gout[:, jj, i * D:(i + 1) * D],
                                        oh[:], start=(j == 0), stop=(j == C - 1),
                                        skip_group_check=True)
                            else:
                                nc.tensor.matmul(
                                    pss[0][:], oh[:], gout[:, jj, :],
                                    start=(j == 0), stop=(j == C - 1))
                            j += 1
                    out_cb(w, pss)

            # ---------- gates ----------
            peT = ap.tile([D, PSH], F32, tag="peT")
            for m in range(NWP):
                tps = pp.tile([D, WIN], F32, name="tp_ps", tag="ps")
                nc.tensor.transpose(tps[:], pe[:, m, :], ident[:WIN, :WIN])
                nc.vector.tensor_copy(peT[:, m * WIN:(m + 1) * WIN], tps[:])

            gates = ap.tile([WIN, NWP, 3 * D], F32)
            for m in range(NWP):
                for g in range(3):
                    gps = pp.tile([WIN, D], F32, name="g_ps", tag="ps")
                    nc.tensor.matmul(gps[:], peT[:, m * WIN:(m + 1) * WIN],
                                     w3[:, g * D:(g + 1) * D], start=True,
                                     stop=True)
                    sig = wp.tile([WIN, D], F32, name="g_sig", tag="gsig")
                    nc.vector.tensor_tensor(
                        sig[:], gps[:], bias[:WIN, g * D:(g + 1) * D], op=OP.add)
                    nc.scalar.activation(sig[:], sig[:], AF.Sigmoid)
                    nc.vector.tensor_tensor(
                        gates[:, m, g * D:(g + 1) * D], sig[:], pe[:, m, :],
                        op=OP.mult)

            gates_full = ag_shard(gates, 3 * D, "gates")

            # ---------- geo chain ----------
            gx1 = ap.tile([WIN, NWP, D], F32)
            spmm("geo", gates_full[:, 0:D], 3 * D,
                 lambda w, pss: nc.vector.tensor_tensor(
                     gx1[:, w, :], pss[0][:], gates[:, w, 0:D], op=OP.add))
            gx1_full = ag_shard(gx1, D, "gx1")

            gx2 = ap.tile([WIN, NWP, D], F32)
            spmm("geo", gx1_full[:], D,
                 lambda w, pss: nc.vector.tensor_tensor(
                     gx2[:, w, :], pss[0][:], gx1[:, w, :], op=OP.add))

            ng = ap.tile([WIN, NWP, D], F32)
            for m in range(NWP):
                t3 = wp.tile([WIN, D], F32, name="gp3", tag="p3")
                nc.vector.tensor_tensor(t3[:], gates[:, m, 0:D], gx1[:, m, :],
                                        op=OP.add)
                nc.vector.tensor_tensor(t3[:], t3[:], gx2[:, m, :], op=OP.add)
                l2norm(ng[:, m, :], t3[:])

            # ---------- seq chain ----------
            sm1 = ap.tile([WIN, NWP, D], F32, tag="smx")
            spmm("tar", gates_full[:, D:2 * D], 3 * D,
                 lambda w, pss: nc.vector.tensor_copy(sm1[:, w, :], pss[0][:]))
            sm1_full = ag_shard(sm1, D, "sm1")

            sx1 = ap.tile([WIN, NWP, D], F32)
            spmm("src", sm1_full[:], D,
                 lambda w, pss: nc.vector.tensor_tensor(
                     sx1[:, w, :], pss[0][:], gates[:, w, D:2 * D], op=OP.add))
            sx1_full = ag_shard(sx1, D, "sx1")

            sm2 = ap.tile([WIN, NWP, D], F32, tag="smx")
            spmm("tar", sx1_full[:], D,
                 lambda w, pss: nc.vector.tensor_copy(sm2[:, w, :], pss[0][:]))
            sm2_full = ag_shard(sm2, D, "sm2")

            sx2 = ap.tile([WIN, NWP, D], F32)
            spmm("src", sm2_full[:], D,
                 lambda w, pss: nc.vector.tensor_tensor(
                     sx2[:, w, :], pss[0][:], sx1[:, w, :], op=OP.add))

            ns = ap.tile([WIN, NWP, D], F32)
            for m in range(NWP):
                t3 = wp.tile([WIN, D], F32, name="sp3", tag="p3")
                nc.vector.tensor_tensor(t3[:], gates[:, m, D:2 * D],
                                        sx1[:, m, :], op=OP.add)
                nc.vector.tensor_tensor(t3[:], t3[:], sx2[:, m, :], op=OP.add)
                l2norm(ns[:, m, :], t3[:])

            # ---------- ng|ns gathers + transposed copy ----------
            ngns = ap.tile([WIN, NWP, 2 * D], F32)
            for m in range(NWP):
                nc.vector.tensor_copy(ngns[:, m, 0:D], ng[:, m, :])
                nc.vector.tensor_copy(ngns[:, m, D:2 * D], ns[:, m, :])
            ngns_full = ag_shard(ngns, 2 * D, "ngns")  # [10000, 128]

            up_table = dr.tile([P, 3 * D], F32)
            nc.sync.dma_start(up_table[:, 0:2 * D], ngns_full[:])
            nc.sync.dma_start(up_table[:, 2 * D:3 * D],
                              gates_full[:, 2 * D:3 * D])

            # transposed shard -> AG -> [128, 10000] in SBUF
            ngnsT = ap.tile([128, PSH], F32, tag="peT")
            for m in range(NWP):
                tps = pp.tile([128, WIN], F32, name="trn_ps", tag="ps")
                nc.tensor.transpose(tps[:], ngns[:, m, :], ident[:WIN, :WIN])
                nc.vector.tensor_copy(ngnsT[:, m * WIN:(m + 1) * WIN], tps[:])
            ngnsT_b = dr.tile([128, PSH], F32)
            nc.sync.dma_start(ngnsT_b[:], ngnsT[:])
            ngnsT_f = dr.tile([128 * NCORES, PSH], F32, addr_space="Shared")
            nc.gpsimd.collective_compute(
                "AllGather", OP.bypass, replica_groups=RG,
                ins=[ngnsT_b.opt()], outs=[ngnsT_f.opt()])
            # full nsT [64, 10000]: rows 64:128 of each rank block, base 0
            nsT = ap.tile([D, P], F32)
            nc.sync.dma_start(
                nsT.rearrange("p (r c) -> p r c", r=NCORES),
                ngnsT_f.rearrange("(r p) c -> p r c", p=128)[D:2 * D])

            # ---------- stage F: up-3in1 (transposed scatter) ----------
            hg = ap.tile([WIN, NWU, D], F32)

            def up_cb(w, pss):
                mT = [wp.tile([D, WIN], F32, name=f"mT{i}", tag=f"mT{i}")
                      for i in range(7)]
                for i in range(3):
                    nc.vector.tensor_copy(mT[i][:], pss[i])
                nc.vector.tensor_tensor(mT[3][:], mT[0][:], mT[1][:], op=OP.mult)
                nc.vector.tensor_tensor(mT[4][:], mT[0][:], mT[2][:], op=OP.mult)
                nc.vector.tensor_tensor(mT[5][:], mT[1][:], mT[2][:], op=OP.mult)
                nc.vector.tensor_tensor(mT[6][:], mT[3][:], mT[2][:], op=OP.mult)
                me_ps = pp.tile([WIN, D], F32, name="me_ps", tag="ps")
                for g in range(7):
                    nc.tensor.matmul(me_ps[:], mT[g][:], fw[:, g, :],
                                     start=(g == 0), stop=(g == 6),
                                     skip_group_check=True)
                me = wp.tile([WIN, D], F32, name="me", tag="me_sb")
                nc.vector.tensor_tensor(me[:], me_ps[:],
                                        bias[:WIN, 3 * D:4 * D], op=OP.add)
                prod = wp.tile([WIN, D], F32, name="prod", tag="prod")
                nc.vector.tensor_tensor(prod[:], me[:], ue[:, w, :], op=OP.mult)
                nc.vector.tensor_tensor(me[:], me[:], ue[:, w, :], op=OP.add)
                nc.vector.tensor_tensor(hg[:, w, :], me[:], prod[:], op=OP.add)

            spmm("up", up_table[:], 3 * D, up_cb, n_d=3 * D, trans3=True)
            hg_full = ag_shard(hg, D, "hg")

            # ---------- pu pass ----------
            fp = ap.tile([WIN, NWP, D], F32)

            def pu_cb(w, pss):
                hgp = wp.tile([WIN, D], F32, name="hgp", tag="hgp")
                nc.vector.tensor_tensor(hgp[:], pss[0][:],
                                        gates[:, w, 2 * D:3 * D], op=OP.add)
                l2norm(fp[:, w, :], hgp[:])
                nc.vector.tensor_tensor(fp[:, w, :], fp[:, w, :], ng[:, w, :],
                                        op=OP.add)
                nc.vector.tensor_tensor(fp[:, w, :], fp[:, w, :], ns[:, w, :],
                                        op=OP.add)

            spmm("pu", hg_full[:], D, pu_cb)
            nc.sync.dma_start(
                t_fp_out.ap().rearrange("(m p) d -> p m d", p=WIN), fp[:])
            fp_full = ag_shard(fp, D, "fp")

            # ---------- final up pass ----------
            un = ap.tile([WIN, NWU, D], F32)

            def upf_cb(w, pss):
                us = wp.tile([WIN, D], F32, name="us", tag="us")
                nc.vector.tensor_copy(us[:], pss[0][:])
                l2norm(un[:, w, :], us[:])

            i_sb, r_sb, v_sb, C = sb_streams["up"]
            nidx = C * 128
            ncall = -(-C // 32)
            for w in range(NWU):
                ps = pp.tile([WIN, D], F32, name="upf_ps", tag="ps")
                j = 0
                for call in range(ncall):
                    cc = min(32, C - call * 32)
                    i0 = (w * nidx + call * 32 * 128) // 16
                    gout = gpool.tile([128, cc, D], F32, name="g_upf", tag="g64")
                    nc.gpsimd.dma_gather(
                        gout[:], fp_full[:], i_sb[:, i0:i0 + cc * 8],
                        cc * 128, cc * 128, D, elem_step=D,
                        single_packet=False, queue_num=(w * ncall + call) % 4)
                    for jj in range(cc):
                        oh = one_hot("upf", r_sb, v_sb, w * C + j)
                        nc.tensor.matmul(ps[:], oh[:], gout[:, jj, :],
                                         start=(j == 0), stop=(j == C - 1))
                        j += 1
                upf_cb(w, [ps])

            un_full = ag_shard(un, D, "un")

            bu = gpool.tile([128, BSH // 128, D], F32, tag="g64")
            nc.gpsimd.dma_gather(bu[:], un_full[:], uidx_sb[:], BSH, BSH, D,
                                 elem_step=D, single_packet=False, queue_num=0)
            nc.sync.dma_start(
                t_bu_out.ap().rearrange("(b p) d -> p b d", p=128), bu[:])

            # ---------- InfoNCE ----------
            NR = NWP
            NCC = -(-P // 512)
            negrow = ap.tile([WIN, NWP], F32)
            nc_b = dr.tile([1, P], F32)
            for n in range(NCC):
                c0, c1 = n * 512, min((n + 1) * 512, P)
                csps = ppz.tile([1, 512], F32, name="cs_ps", tag="cs", bufs=1)
                for m in range(NR):
                    zps = ppz.tile([WIN, 512], F32, name="z_ps", tag="z")
                    nc.tensor.matmul(
                        zps[:, :c1 - c0], ngnsT[0:D, m * WIN:(m + 1) * WIN],
                        nsT[:, c0:c1], start=True, stop=True,
                        skip_group_check=True)
                    ex = wp.tile([WIN, 512], F32, name="ex", tag="ex")
                    rs = wp.tile([WIN, 1], F32, name="rs", tag="rs")
                    nc.scalar.activation(ex[:, :c1 - c0], zps[:, :c1 - c0],
                                         AF.Exp, scale=1.0 / TEMP,
                                         accum_out=rs[:])
                    if n == 0:
                        nc.vector.tensor_copy(negrow[:, m:m + 1], rs[:])
                    else:
                        nc.vector.tensor_tensor(negrow[:, m:m + 1],
                                                negrow[:, m:m + 1], rs[:],
                                                op=OP.add)
                    nc.tensor.matmul(csps[:, :c1 - c0], ones[:WIN, :],
                                     ex[:, :c1 - c0], start=(m == 0),
                                     stop=(m == NR - 1), skip_group_check=True)
                cs_sb = wp.tile([1, 512], F32, name="cs_sb", tag="cssb")
                nc.vector.tensor_copy(cs_sb[:, :c1 - c0], csps[:, :c1 - c0])
                nc.sync.dma_start(nc_b[:, c0:c1], cs_sb[:, :c1 - c0])

            nc_rs = dr.tile([1, PSH], F32)
            nc.gpsimd.collective_compute("ReduceScatter", OP.add,
                                         replica_groups=RG,
                                         ins=[nc_b.opt()], outs=[nc_rs.opt()])
            negcol_sh = ap.tile([WIN, NWP], F32)
            nc.sync.dma_start(negcol_sh[:],
                              nc_rs.rearrange("o (m p) -> p (o m)", p=WIN))

            pos = ap.tile([WIN, NWP], F32)
            for m in range(NWP):
                dot = wp.tile([WIN, 1], F32, name="dot", tag="dot")
                tmp = wp.tile([WIN, D], F32, name="dtmp", tag="dtmp")
                nc.vector.scalar_tensor_tensor(tmp[:], ng[:, m, :], 1.0,
                                               ns[:, m, :], OP.mult, OP.mult,
                                               accum_out=dot[:])
                nc.scalar.activation(pos[:, m:m + 1], dot[:], AF.Exp,
                                     scale=1.0 / TEMP)

            def loss_terms(dst, neg):
                r = wp.tile([WIN, NWP], F32, name="lt_r", tag="ltr")
                nc.vector.tensor_scalar_add(r[:], neg[:], 1e-8)
                nc.vector.reciprocal(r[:], r[:])
                nc.vector.tensor_tensor(r[:], r[:], pos[:], op=OP.mult)
                nc.vector.tensor_scalar_add(r[:], r[:], 1e-8)
                nc.scalar.activation(r[:], r[:], AF.Ln)
                sps = pp.tile([1, NWP], F32, name="ls_ps", tag="ps")
                nc.tensor.matmul(sps[:], ones[:WIN, :], r[:], start=True,
                                 stop=True, skip_group_check=True)
                nc.vector.tensor_reduce(dst, sps[:], axis=mybir.AxisListType.X,
                                        op=OP.add)

            lsum = ap.tile([1, 2], F32)
            loss_terms(lsum[:, 0:1], negrow)
            loss_terms(lsum[:, 1:2], negcol_sh)

            ls_b = dr.tile([1, 2], F32)
            nc.sync.dma_start(ls_b[:], lsum[:])
            ls_ar = dr.tile([1, 2], F32, addr_space="Shared")
            nc.gpsimd.collective_compute("AllReduce", OP.add, replica_groups=RG,
                                         ins=[ls_b.opt()], outs=[ls_ar.opt()])
            ls_sb = wp.tile([1, 2], F32, name="ls_sb", tag="lssb")
            nc.sync.dma_start(ls_sb[:], ls_ar[:])
            loss = wp.tile([1, 1], F32, name="loss", tag="loss")
            nc.vector.tensor_reduce(loss[:], ls_sb[:], axis=mybir.AxisListType.X,
                                    op=OP.add)
            nc.vector.tensor_scalar_mul(loss[:], loss[:], -0.5 / P)
            nc.sync.dma_start(t_loss_out[:], loss[:])

    nc.compile()
    return nc


_CACHE = {}


def kernel(**inputs):
    poi_emb = np.asarray(inputs["poi_emb"], np.float32)
    user_emb = np.asarray(inputs["user_emb"], np.float32)
    pe_full = poi_emb[:P]

    Cg, gi, gr, gv = prep_matrix(np.asarray(inputs["geo_rows"]),
                                 np.asarray(inputs["geo_cols"]),
                                 np.asarray(inputs["geo_vals"], np.float32), P, NWP)
    Ct, ti, tr, tv = prep_matrix(np.asarray(inputs["tar_rows"]),
                                 np.asarray(inputs["tar_cols"]),
                                 np.asarray(inputs["tar_vals"], np.float32), P, NWP)
    Cs, si, sr, sv = prep_matrix(np.asarray(inputs["src_rows"]),
                                 np.asarray(inputs["src_cols"]),
                                 np.asarray(inputs["src_vals"], np.float32), P, NWP)
    Cu, ui, ur, uv = prep_matrix(np.asarray(inputs["up_rows"]),
                                 np.asarray(inputs["up_cols"]),
                                 np.asarray(inputs["up_vals"], np.float32), U, NWU)
    Cp, pi, pr, pv = prep_matrix(np.asarray(inputs["pu_rows"]),
                                 np.asarray(inputs["pu_cols"]),
                                 np.asarray(inputs["pu_vals"], np.float32), P, NWP)

    meta = dict(Cg=Cg, Ct=Ct, Cs=Cs, Cu=Cu, Cp=Cp)
    key = tuple(sorted(meta.items()))
    if key not in _CACHE:
        _CACHE[key] = build(meta)
    nc = _CACHE[key]

    w3 = np.concatenate([np.asarray(inputs["w_gate_geo"], np.float32),
                         np.asarray(inputs["w_gate_seq"], np.float32),
                         np.asarray(inputs["w_gate_col"], np.float32)], axis=1)
    biases = np.concatenate(
        [np.tile(np.asarray(inputs[k], np.float32).reshape(1, D), (128, 1))
         for k in ("b_gate_geo", "b_gate_seq", "b_gate_col", "fusion_b")],
        axis=1)
    iota = np.tile(np.arange(128, dtype=np.float32).reshape(1, 128), (128, 1))
    pidx_a = np.arange(128, dtype=np.float32).reshape(128, 1)
    ones = np.ones((128, 1), np.float32)
    uidx = np.asarray(inputs["user_idx"]).reshape(NCORES, BSH)

    in_maps = []
    for c in range(NCORES):
        in_maps.append({
            "pe_shard": np.ascontiguousarray(pe_full[c * PSH:(c + 1) * PSH]),
            "w3": w3,
            "fusion_w": np.ascontiguousarray(
                np.asarray(inputs["fusion_w"], np.float32)
                .reshape(7, D, D).transpose(1, 0, 2).reshape(D, 7 * D)),
            "biases": biases, "iota": iota, "pidx": pidx_a, "ones": ones,
            "ue_shard": np.ascontiguousarray(user_emb[c * USH:(c + 1) * USH]),
            "uidx": _wrap16(uidx[c]),
            "geo_idx": gi[c], "geo_rloc": gr[c], "geo_val": gv[c],
            "tar_idx": ti[c], "tar_rloc": tr[c], "tar_val": tv[c],
            "src_idx": si[c], "src_rloc": sr[c], "src_val": sv[c],
            "up_idx": ui[c], "up_rloc": ur[c], "up_val": uv[c],
            "pu_idx": pi[c], "pu_rloc": pr[c], "pu_val": pv[c],
        })

    res = bass_utils.run_bass_kernel_spmd(nc, in_maps,
                                          core_ids=list(range(NCORES)))
    fp = np.concatenate([res.results[c]["fusion_pois"] for c in range(NCORES)],
                        axis=0)
    bu = np.concatenate([res.results[c]["batch_users"] for c in range(NCORES)],
                        axis=0)
    loss = np.float32(res.results[0]["loss"][0, 0])
    return bu, fp, loss
